# revision 38
# baseline (speedup 1.0000x reference)
"""Expert-parallel MoE SwiGLU kernel for 8 Trainium2 NeuronCores.

Problem: N=4096 tokens, top-2 of E=8 experts, H=2048, I=1408, fp32.

Strategy (load-balanced expert parallel):
  - Host-side dispatch: sort routed (token, k) slots by expert, then pack
    them into 16 fixed-capacity column slots -- each core gets one A-slot
    (cap `a`) and one B-slot (cap `b`), each slot holding tokens of a
    single expert.  Caps (a, b) are solved per routing instance so that
    a + b ~= max-balanced capacity (1064 here vs 1152 for naive
    one-expert-per-core), with heavy experts split across two A-slots and
    light experts across two B-slots.
  - Device (same SPMD program on all 8 cores): each core computes
        y^T[:, 0:a]   = Wd_A @ (silu(Wg_A @ x^T[:, 0:a]) * (Wu_A @ x^T[:, 0:a]))
        y^T[:, a:a+b] = Wd_B @ (...same with B weights...)
    entirely in [feature, token] layout.  Matmuls run in bf16 (single-pass
    PE rate; abs-max rel err vs fp32 reference ~4e-3).
  - Host-side combine: weighted scatter-add of per-slot outputs back to
    the [N, H] output.

The NEFF is compiled per (a, b) capacity pair (cached within process);
compile time is host-side and does not count toward HW exec time.
"""

import numpy as np

import concourse.bass as bass
import concourse.tile as tile
from concourse import bacc, mybir
from concourse import bass_utils
from concourse.tile import add_dep_helper

N, K, E, H, I = 4096, 2, 8, 2048, 1408
P = 128
HCH = H // P   # 16 chunks over hidden dim
ICH = I // P   # 11 chunks over intermediate dim
F32 = mybir.dt.float32
F32R = mybir.dt.float32r
BF16 = mybir.dt.bfloat16


def _r8(v):
    return -(-int(v) // 8) * 8


def _r2(v):
    return -(-int(v) // 2) * 2


def _split_cols(width, base=0):
    """Split width into n<=ceil(width/512) chunks each <=512, mult-of-8-ish.
    Returns [(abs_off, w), ...] offset by `base`."""
    n = -(-width // 512)
    out, off = [], 0
    for j in range(n):
        w = _r8((width - off) // (n - j)) if j < n - 1 else width - off
        out.append((base + off, w))
        off += w
    assert off == width and all(w <= 512 for _, w in out)
    return out


def _build(a, b, xdt=BF16, wdt=BF16, hdt=BF16):
    """Build + compile the per-core 2-segment SwiGLU kernel."""
    C = a + b
    cha = _split_cols(a, 0)        # segment A chunks (absolute col ranges)
    chb = _split_cols(b, a)        # segment B chunks
    allch = cha + chb
    assert len(cha) * 2 + len(chb) * 2 <= 8          # phase-1 PSUM banks
    assert len(allch) * 2 <= 8                       # phase-2 PSUM banks

    nc = bacc.Bacc("TRN2", target_bir_lowering=False, debug=False,
                   enable_asserts=False, num_devices=E)

    xT = nc.dram_tensor("xT", [H, C], xdt, kind="ExternalInput")
    # weights come host-pre-tiled so every DMA line is contiguous:
    # wg[i, p, h*128+j] = Wg[e][i*128+j, h*128+p]  (lhsT tiles back to back)
    wseg = []
    for s in ("A", "B"):
        wseg.append((
            nc.dram_tensor(f"wg{s}", [ICH, P, H], wdt, kind="ExternalInput"),
            nc.dram_tensor(f"wu{s}", [ICH, P, H], wdt, kind="ExternalInput"),
            nc.dram_tensor(f"wd{s}", [HCH, P, I], wdt, kind="ExternalInput"),
        ))
    outT = nc.dram_tensor("outT", [H, C], F32, kind="ExternalOutput")

    x_r = xT.ap().rearrange("(ho p) c -> p ho c", p=P)      # [128, 16, C]
    out_r = outT.ap().rearrange("(ho p) c -> p ho c", p=P)  # [128, 16, C]

    with tile.TileContext(nc) as tc:
        with (
            tc.tile_pool(name="xpool", bufs=1) as xpool,
            tc.tile_pool(name="hpool", bufs=1) as hpool,
            tc.tile_pool(name="wpool", bufs=2) as wpool,
            tc.tile_pool(name="wpoolB", bufs=3) as wpoolB,
            tc.tile_pool(name="dpool", bufs=4) as dpool,
            tc.tile_pool(name="opool", bufs=2) as opool,
        ):
            x_sb = xpool.tile([P, HCH, C], xdt)
            hid_sb = hpool.tile([P, ICH, C], hdt)

            # ---- front choreography ------------------------------------
            # Segment B (the smaller cap) is processed first, so only
            # x[:, a:C] plus B's i=0 gate/up weights are front-critical
            # (~3MB).  DMA facts learned from traces:
            #   - a DMA issue costs ~0.65us of engine-queue time, so front
            #     pieces must be few and large, split across two HWDGE
            #     rings (sync: i=0 weights + first x piece, in consumption
            #     order of the m-interleaved i=0 group; scalar: bulk x);
            #   - the gpsimd(SWDGE) ring is 16 parallel per-engine queues
            #     with only ~30GB/s each -- no use for bulk data;
            #   - the Tile scheduler hoists any ungated DMA to t=0, so
            #     everything not wanted in the front needs an explicit dep.
            wgA_r, wuA_r, wdA_r = wseg[0][0].ap(), wseg[0][1].ap(), wseg[0][2].ap()
            wgB_r, wuB_r, wdB_r = wseg[1][0].ap(), wseg[1][1].ap(), wseg[1][2].ap()
            wB0 = wpoolB.tile([P, 2, H], wdt, tag="wB", name="wB_0")
            nc.sync.dma_start(wB0[:, 0, 0:2 * P], wgB_r[0][:, 0:2 * P])
            nc.sync.dma_start(wB0[:, 1, 0:2 * P], wuB_r[0][:, 0:2 * P])
            nc.sync.dma_start(x_sb[:, 0:2, a:C], x_r[:, 0:2, a:C])
            nc.sync.dma_start(wB0[:, 0, 2 * P:10 * P], wgB_r[0][:, 2 * P:10 * P])
            nc.sync.dma_start(wB0[:, 1, 2 * P:10 * P], wuB_r[0][:, 2 * P:10 * P])
            nc.sync.dma_start(wB0[:, 0, 10 * P:], wgB_r[0][:, 10 * P:])
            nc.sync.dma_start(wB0[:, 1, 10 * P:], wuB_r[0][:, 10 * P:])
            xb = {}
            xb[0] = nc.scalar.dma_start(x_sb[:, 2:6, a:C], x_r[:, 2:6, a:C])
            xb[1] = nc.scalar.dma_start(x_sb[:, 6:11, a:C], x_r[:, 6:11, a:C])
            xb[2] = nc.scalar.dma_start(x_sb[:, 11:16, a:C], x_r[:, 11:16, a:C])

            # ---- phase 1: gate/up + SwiGLU per segment -> hidden^T -----
            # The Tile scheduler hoists any DMA with no dependencies to the
            # start, so every first-buffer DMA (before pool WAR pacing kicks
            # in) is explicitly gated to keep the front HBM window clean.
            wb_d1 = {}            # seg-B weight DMAs, for pacing x-A / wA
            wa_last = []          # seg-A weight DMAs, for pacing wd below
            with tc.tile_pool(name="ps1", bufs=1, space="PSUM") as ps1:
                for seg, (ch, wg_r, wu_r, stag) in enumerate(
                        ((chb, wgB_r, wuB_r, "B"), (cha, wgA_r, wuA_r, "A"))):
                    for i in range(ICH):
                        if seg == 0 and i == 0:
                            w_sb = wB0
                        else:
                            pool = wpoolB if seg == 0 else wpool
                            w_sb = pool.tile([P, 2, H], wdt, tag=f"w{stag}",
                                             name=f"w{stag}_{i}")
                            d0 = nc.sync.dma_start(w_sb[:, 0], wg_r[i])
                            d1 = nc.sync.dma_start(w_sb[:, 1], wu_r[i])
                            if seg == 0 and i == 1:
                                # stream i=1 weights concurrent with the
                                # last x-B chunks so i=1 starts stall-free
                                add_dep_helper(d0.ins, xb[0].ins,
                                               reason="pace wB1 into front tail")
                                add_dep_helper(d1.ins, xb[0].ins,
                                               reason="pace wB1 into front tail")
                            if seg == 0 and i == 2:
                                # wB2 has a fresh buffer (bufs=3, no WAR) --
                                # gate it so it is not hoisted into the front
                                add_dep_helper(d0.ins, wb_d1[1].ins,
                                               reason="pace wB2 behind wB1")
                                add_dep_helper(d1.ins, wb_d1[1].ins,
                                               reason="pace wB2 behind wB1")
                            if seg == 1 and i < 2:
                                # first wA buffers have no pool predecessor
                                add_dep_helper(d0.ins, wb_d1[5 + i].ins,
                                               reason="pace wA behind wB stream")
                                add_dep_helper(d1.ins, wb_d1[5 + i].ins,
                                               reason="pace wA behind wB stream")
                            if seg == 0:
                                wb_d1[i] = d1
                            else:
                                wa_last.append(d1)
                        if seg == 0 and i == 3:
                            # x-A held behind the early wB stream (needed
                            # only when phase 1's A segment starts)
                            xa0 = nc.scalar.dma_start(x_sb[:, 0:8, 0:a],
                                                      x_r[:, 0:8, 0:a])
                            add_dep_helper(xa0.ins, wb_d1[1].ins,
                                           reason="yield front BW")
                            xa1 = nc.scalar.dma_start(x_sb[:, 8:16, 0:a],
                                                      x_r[:, 8:16, 0:a])
                            add_dep_helper(xa1.ins, wb_d1[2].ins,
                                           reason="yield front BW")
                        ps_g = [
                            ps1.tile([P, cw], F32, name=f"psg{stag}_{i}_{n}",
                                     tag=f"psg{stag}{n}")
                            for n, (c0, cw) in enumerate(ch)
                        ]
                        ps_u = [
                            ps1.tile([P, cw], F32, name=f"psu{stag}_{i}_{n}",
                                     tag=f"psu{stag}{n}")
                            for n, (c0, cw) in enumerate(ch)
                        ]
                        # Only the first segment's i=0 interleaves gate/up
                        # per h (widens the front x-deadline to the whole
                        # i=0 span); all other groups keep m-outer order so
                        # the gate psum stop lands mid-group and the silu
                        # read never stalls the next group's first matmul.
                        if seg == 0 and i == 0:
                            mh_iter = [(h, m) for h in range(HCH)
                                       for m in (0, 1)]
                        else:
                            mh_iter = [(h, m) for m in (0, 1)
                                       for h in range(HCH)]
                        last_group = (seg == 1 and i == ICH - 1)
                        for h, m in mh_iter:
                            ps = ps_g if m == 0 else ps_u
                            lhsT = w_sb[:, m, h * P:(h + 1) * P]
                            for n, (c0, cw) in enumerate(ch):
                                if last_group and m == 1:
                                    continue      # emitted chunk-major below
                                nc.tensor.matmul(
                                    ps[n][:],
                                    lhsT,
                                    x_sb[:, h, c0:c0 + cw],
                                    start=(h == 0),
                                    stop=(h == HCH - 1),
                                )
                        if last_group:
                            # final group: run the up sweep chunk-major so
                            # the first chunk's psum stop (and its mul) land
                            # early -- shortens the phase-2 entry wait on
                            # the ps1 pool close.
                            for n, (c0, cw) in enumerate(ch):
                                for h in range(HCH):
                                    nc.tensor.matmul(
                                        ps_u[n][:],
                                        w_sb[:, 1, h * P:(h + 1) * P],
                                        x_sb[:, h, c0:c0 + cw],
                                        start=(h == 0),
                                        stop=(h == HCH - 1),
                                    )
                        for n, (c0, cw) in enumerate(ch):
                            hs = hid_sb[:, i, c0:c0 + cw]
                            nc.scalar.activation(
                                out=hs, in_=ps_g[n][:],
                                func=mybir.ActivationFunctionType.Silu,
                            )
                            nc.vector.tensor_mul(out=hs, in0=hs, in1=ps_u[n][:])

            # ---- phase 2: down projection -> out^T [H, C] --------------
            with tc.tile_pool(name="ps2", bufs=2, space="PSUM") as ps2:
                for h in range(HCH):
                    wd_sb = {}
                    for seg, wd_r in ((0, wdA_r), (1, wdB_r)):
                        stag = "AB"[seg]
                        t = dpool.tile([P, I], wdt, tag=f"wd{stag}")
                        dd = nc.gpsimd.dma_start(t[:], wd_r[h])
                        # SWDGE queues are parallel, so every wd DMA must be
                        # gated individually or it floods the front/phase-1
                        # weight stream; hold them until the wA stream is
                        # fully delivered (wd is not needed until phase 2).
                        add_dep_helper(dd.ins, wa_last[-1].ins,
                                       reason="pace wd behind phase-1 weights")
                        wd_sb[seg] = t
                    # B chunk first: its psum chain stops earliest, so its
                    # copy (scalar engine) overlaps A's remaining matmuls
                    # and the single whole-row output DMA launches sooner.
                    p2ch = ([(c0, cw, 1) for c0, cw in chb]
                            + [(c0, cw, 0) for c0, cw in cha])
                    ps_d = [
                        ps2.tile([P, cw], F32, name=f"psd_{h}_{n}", tag=f"psd{n}")
                        for n, (c0, cw, s) in enumerate(p2ch)
                    ]
                    for i in range(ICH):
                        for n, (c0, cw, s) in enumerate(p2ch):
                            nc.tensor.matmul(
                                ps_d[n][:],
                                wd_sb[s][:, i * P:(i + 1) * P],
                                hid_sb[:, i, c0:c0 + cw],
                                start=(i == 0),
                                stop=(i == ICH - 1),
                            )
                    o_sb = opool.tile([P, C], F32, tag="o")
                    for n, (c0, cw, s) in enumerate(p2ch):
                        if s == 1:
                            nc.scalar.activation(
                                out=o_sb[:, c0:c0 + cw], in_=ps_d[n][:],
                                func=mybir.ActivationFunctionType.Copy,
                            )
                        else:
                            nc.vector.tensor_copy(o_sb[:, c0:c0 + cw],
                                                  ps_d[n][:])
                    if h == HCH - 1:
                        # tail: ship B cols (copied first, on scalar) while
                        # the A-col copies finish, then the A cols
                        nc.sync.dma_start(out_r[:, h, a:C], o_sb[:, a:C])
                        nc.sync.dma_start(out_r[:, h, 0:a], o_sb[:, 0:a])
                    else:
                        nc.sync.dma_start(out_r[:, h, :], o_sb[:])

    nc.compile()
    return nc


_NC_CACHE = {}

# compute dtype config: "f32r" (FP22 single-pass, ~3e-4 rel err) or "bf16"
DTYPES = {
    "f32r": (F32R, F32R, F32R),
    "bf16": (BF16, BF16, BF16),
}
import os
CONFIG = os.environ.get("MOE_KERNEL_CONFIG", "bf16")


def _get_nc(a, b):
    key = (a, b, CONFIG)
    if key not in _NC_CACHE:
        _NC_CACHE[key] = _build(a, b, *DTYPES[CONFIG])
    return _NC_CACHE[key]


def _solve_caps(counts):
    """Pick per-core segment caps (a, b) and an expert->slots assignment.

    Slots: 8 A-slots (cap a, one per core) + 8 B-slots (cap b).  Every
    expert is covered by exactly two slots: heavy experts by 2 A-slots,
    middling by A+B, light by 2 B-slots.  Minimizes a+b.
    Returns (a, b, plan) with plan = list of (expert, [slot, ...]) where
    slot = (core, seg).  Falls back to one-expert-per-core if infeasible.
    """
    counts = np.asarray(counts)
    order = np.argsort(-counts, kind="stable")
    c = counts[order]
    best = None
    for m2 in (0, 2, 4, 6, 8):          # |M| = experts using one A + one B
        g = (8 - m2) // 2               # |A2| = |B2|
        if g == 0:
            Cc = _r8(c[0])
            aa = _r8(-(-c[0] // 2))
            bb = Cc - aa
        else:
            maxA2 = c[:g].max()
            maxB2 = c[g + m2:].max()
            aa = _r8(-(-maxA2 // 2))
            bb = _r8(-(-maxB2 // 2))
            if m2:
                bb = max(bb, _r8(c[g:g + m2].max() - aa))
            Cc = aa + bb
        if bb < 1 or aa < bb:
            continue
        if best is None or Cc < best[0]:
            best = (Cc, aa, bb, m2, g)
    _, a, b, m2, g = best
    # build assignment
    a_slots = [(core, 0) for core in range(8)]
    b_slots = [(core, 1) for core in range(8)]
    plan = []
    for j, e in enumerate(order):
        if j < g:                        # heavy: two A-slots
            plan.append((e, [a_slots.pop(0), a_slots.pop(0)]))
        elif j < g + m2:                 # middling: A + B
            plan.append((e, [a_slots.pop(0), b_slots.pop(0)]))
        else:                            # light: two B-slots
            plan.append((e, [b_slots.pop(0), b_slots.pop(0)]))
    # verify coverage
    for e, slots in plan:
        cap = sum(a if s == 0 else b for _, s in slots)
        assert cap >= counts[e], (e, counts[e], cap, a, b)
    return a, b, plan


def kernel(x, topk_ids, topk_weight, Wg, Wu, Wd):
    x = np.asarray(x, dtype=np.float32)
    topk_ids = np.asarray(topk_ids)
    topk_weight = np.asarray(topk_weight, dtype=np.float32)

    # ---- host-side dispatch (the all-to-all by topk_ids)
    flat = topk_ids.reshape(-1).astype(np.int64)
    order = np.argsort(flat, kind="stable")
    counts = np.bincount(flat, minlength=E)
    toks = order // K          # token index per sorted slot
    ks = order % K             # which of the top-k slots
    bounds = np.cumsum(counts)
    starts = bounds - counts

    a, b, plan = _solve_caps(counts)
    C = a + b
    nc = _get_nc(a, b)

    import ml_dtypes
    xdt, wdt, _ = DTYPES[CONFIG]
    np_x = ml_dtypes.bfloat16 if xdt == BF16 else np.float32
    np_w = ml_dtypes.bfloat16 if wdt == BF16 else np.float32

    def pack_gu(w):  # [I, H] -> [ICH, P, H]; out[i, p, h*128+j] = w[i*128+j, h*128+p]
        v = np.asarray(w, np.float32).reshape(ICH, P, HCH, P)       # [i, j, h, p]
        return np.ascontiguousarray(
            v.transpose(0, 3, 2, 1).astype(np_w)).reshape(ICH, P, H)

    def pack_d(w):   # [H, I] -> [HCH, P, I]; out[h, p, i*128+j] = w[h*128+j, i*128+p]
        v = np.asarray(w, np.float32).reshape(HCH, P, ICH, P)       # [h, j, i, p]
        return np.ascontiguousarray(
            v.transpose(0, 3, 2, 1).astype(np_w)).reshape(HCH, P, I)

    packed = {}

    def get_packed(e):
        if e not in packed:
            packed[e] = (pack_gu(Wg[e]), pack_gu(Wu[e]), pack_d(Wd[e]))
        return packed[e]

    # fill slots with each expert's routed tokens, in slot order
    core_slots = [[None, None] for _ in range(E)]   # [(expert, toks, ks)]
    for e, slots in plan:
        te = toks[starts[e]:bounds[e]]
        ke = ks[starts[e]:bounds[e]]
        off = 0
        for core, seg in slots:
            cap = a if seg == 0 else b
            n = min(cap, len(te) - off)
            core_slots[core][seg] = (e, te[off:off + n], ke[off:off + n])
            off += n
        assert off == len(te)

    in_maps = []
    for core in range(E):
        xT_c = np.zeros((H, C), np_x)
        m = {"xT": xT_c}
        for seg, name in ((0, "A"), (1, "B")):
            e, te, ke = core_slots[core][seg]
            base = 0 if seg == 0 else a
            if len(te):
                xT_c[:, base:base + len(te)] = x[te].T.astype(np_x)
            pg, pu, pd = get_packed(e)
            m[f"wg{name}"], m[f"wu{name}"], m[f"wd{name}"] = pg, pu, pd
        in_maps.append(m)

    res = bass_utils.run_bass_kernel_spmd(nc, in_maps, core_ids=list(range(E)))

    # ---- host-side combine (weighted scatter-add)
    out = np.zeros((N, H), np.float32)
    for core in range(E):
        yT = res.results[core]["outT"]
        for seg in (0, 1):
            e, te, ke = core_slots[core][seg]
            if len(te) == 0:
                continue
            base = 0 if seg == 0 else a
            y = yT[:, base:base + len(te)]                # [H, n]
            w = topk_weight[te, ke].astype(np.float32)
            out[te] += (y * w[None, :]).T
    return out
